# revision 1
# baseline (speedup 1.0000x reference)
"""MSE + SSIM combined loss on Trainium2, data-parallel over 8 NeuronCores.

Reference computes, over [64,3,512,512] f32 inputs:
    loss = 0.7*mean((x-y)^2) + 0.3*(1 - mean(ssim_map(x, y)))
with an 11x11 gaussian (sigma=1.5) depthwise conv, zero-padded (pad=5).

Strategy per core (8 images = 24 channel-images of [512,512]):
  - fields: xb, yb (bf16 casts), s = x^2+y^2, xy
  - separable gaussian conv as two banded matmuls on TensorE:
      d1 (h-conv, transposing): out1T[w,h] = sum_h' X[h',w] g[h-h']
         via matmul(lhsT=X_tile[h',w_blk], rhs=G[h'_tile, h_band])
      d2 (w-conv): out2[w_out,h] = sum_w' g[w_out-w'] out1T[w',h]
         via matmul(lhsT=G[w'_tile, w_out_blk], rhs=out1T[w'_tile, h])
    (G is the symmetric banded [512,512] matrix G[i,j] = g1d[j-i+5];
     the xy field uses 2G in d2 so its conv output is conv(2xy).)
  - elementwise ssim map on DVE/ACT in bf16
  - sums (ssim, s, xy) via ones-vector matmul accumulated in PSUM
  - MSE = sum(s) - 2*sum(xy); host combines per-core partials
"""

import numpy as np
from contextlib import ExitStack

import concourse.bass as bass
import concourse.bacc as bacc
import concourse.mybir as mybir
from concourse import tile
from concourse.bass_utils import run_bass_kernel_spmd

F32 = mybir.dt.float32
BF16 = mybir.dt.bfloat16
AF = mybir.ActivationFunctionType
ALU = mybir.AluOpType

# ---- problem constants (hardcoded; kernel.py must be self-contained) ----
WIN = 11
SIGMA = 1.5
PAD = WIN // 2
DATA_RANGE = 2.0
MSE_W = 0.7
SSIM_W = 0.3
C1 = (0.01 * DATA_RANGE) ** 2
C2 = (0.03 * DATA_RANGE) ** 2

B, C, H, W = 64, 3, 512, 512
NCORES = 8
NIMG = (B // NCORES) * C      # 24 channel-images per core
NT = H // 128                 # 4 tiles per image dim


def _gauss1d():
    coords = np.arange(WIN, dtype=np.float64) - (WIN - 1) / 2.0
    g = np.exp(-(coords ** 2) / (2.0 * SIGMA ** 2))
    return (g / g.sum()).astype(np.float32)


def _band_matrix():
    """G[i, j] = g1d[j - i + PAD] for |j-i|<=PAD else 0  (512x512 f32)."""
    g = _gauss1d()
    G = np.zeros((H, H), dtype=np.float32)
    for d in range(-PAD, PAD + 1):
        np.fill_diagonal(G[max(0, -d):, max(0, d):], g[d + PAD])
    return G


def _band(k):
    """Nonzero output-column range of G rows [128k, 128k+128)."""
    return max(0, 128 * k - PAD), min(H, 128 * (k + 1) + PAD)


def build_nc():
    nc = bacc.Bacc("TRN2")
    x_ext = nc.declare_dram_parameter("x", [NIMG, H, W], F32, isOutput=False)
    y_ext = nc.declare_dram_parameter("y", [NIMG, H, W], F32, isOutput=False)
    g_ext = nc.declare_dram_parameter("g", [H, H], F32, isOutput=False)
    # row 0: sum(ssim) partials, row 1: sum(s), row 2: sum(xy)
    out_ext = nc.declare_dram_parameter("out", [3, H], F32, isOutput=True)

    with ExitStack() as ctx:
        tc = ctx.enter_context(tile.TileContext(nc))
        const_pool = ctx.enter_context(tc.tile_pool(name="const", bufs=1))
        in_pool = ctx.enter_context(tc.tile_pool(name="inp", bufs=6))
        fld_pool = ctx.enter_context(tc.tile_pool(name="fld", bufs=2))
        o1_pool = ctx.enter_context(tc.tile_pool(name="o1", bufs=2))
        tmp_pool = ctx.enter_context(tc.tile_pool(name="tmp", bufs=2))
        ps1_pool = ctx.enter_context(tc.tile_pool(name="ps1", bufs=1, space="PSUM"))
        ps2_pool = ctx.enter_context(tc.tile_pool(name="ps2", bufs=1, space="PSUM"))
        acc_pool = ctx.enter_context(tc.tile_pool(name="acc", bufs=1, space="PSUM"))

        # ---- constants ----
        Gsb, G2sb = [], []
        for k in range(NT):
            gtmp = in_pool.tile([128, H], F32, tag=f"gtmp{k}")
            nc.gpsimd.dma_start(gtmp[:], g_ext[128 * k:128 * (k + 1), :])
            gk = const_pool.tile([128, H], BF16, tag=f"g{k}")
            nc.scalar.copy(gk[:], gtmp[:])
            g2k = const_pool.tile([128, H], BF16, tag=f"g2{k}")
            nc.scalar.mul(g2k[:], gtmp[:], 2.0)
            Gsb.append(gk)
            G2sb.append(g2k)
        ones = const_pool.tile([128, 1], BF16, tag="ones")
        nc.gpsimd.memset(ones[:], 1.0)

        # ---- PSUM sum accumulators (ones-vector matmul targets) ----
        acc_ssim = acc_pool.tile([1, H], F32, tag="acc_ssim")
        acc_s = acc_pool.tile([1, H], F32, tag="acc_s")
        acc_xy = acc_pool.tile([1, H], F32, tag="acc_xy")
        n_ssim = NIMG * NT
        n_fld = NIMG * NT
        i_ssim = i_s = i_xy = 0

        for i in range(NIMG):
            # ---- load + field prep ----
            xb, yb, s, xy = [], [], [], []
            for t in range(NT):
                xf = in_pool.tile([128, W], F32, tag="xf")
                nc.gpsimd.dma_start(xf[:], x_ext[i, 128 * t:128 * (t + 1), :])
                yf = in_pool.tile([128, W], F32, tag="yf")
                nc.gpsimd.dma_start(yf[:], y_ext[i, 128 * t:128 * (t + 1), :])

                xbt = fld_pool.tile([128, W], BF16, tag=f"xb{t}")
                nc.vector.tensor_copy(xbt[:], xf[:])
                ybt = fld_pool.tile([128, W], BF16, tag=f"yb{t}")
                nc.vector.tensor_copy(ybt[:], yf[:])

                x2t = tmp_pool.tile([128, W], BF16, tag="x2")
                nc.scalar.activation(x2t[:], xbt[:], AF.Square)
                y2t = tmp_pool.tile([128, W], BF16, tag="y2")
                nc.scalar.activation(y2t[:], ybt[:], AF.Square)

                st = fld_pool.tile([128, W], BF16, tag=f"s{t}")
                nc.vector.tensor_tensor(st[:], x2t[:], y2t[:], ALU.add)
                xyt = fld_pool.tile([128, W], BF16, tag=f"xy{t}")
                nc.vector.tensor_tensor(xyt[:], xbt[:], ybt[:], ALU.mult)

                nc.tensor.matmul(acc_s[:, :], lhsT=ones[:], rhs=st[:],
                                 start=(i_s == 0), stop=(i_s == n_fld - 1),
                                 skip_group_check=True)
                i_s += 1
                nc.tensor.matmul(acc_xy[:, :], lhsT=ones[:], rhs=xyt[:],
                                 start=(i_xy == 0), stop=(i_xy == n_fld - 1),
                                 skip_group_check=True)
                i_xy += 1

                xb.append(xbt); yb.append(ybt); s.append(st); xy.append(xyt)

            fields = [xb, yb, s, xy]

            # ---- d1: h-conv, transposing.  o1[f][wb] = [w_blk, h] bf16 ----
            o1 = [[None] * NT for _ in range(4)]
            for f in range(4):
                for wb in range(NT):
                    ps1 = ps1_pool.tile([128, H], F32, tag="ps1")
                    for k in range(NT):
                        lo, hi = _band(k)
                        nc.tensor.matmul(
                            ps1[:, lo:hi],
                            lhsT=fields[f][k][:, 128 * wb:128 * (wb + 1)],
                            rhs=Gsb[k][:, lo:hi],
                            start=(k == 0), stop=(k == NT - 1),
                            skip_group_check=True)
                    o1t = o1_pool.tile([128, H], BF16, tag=f"o1_{f}_{wb}")
                    nc.vector.tensor_copy(o1t[:], ps1[:])
                    o1[f][wb] = o1t

            # ---- d2: w-conv on transposed data + ssim elementwise ----
            for wb in range(NT):
                ps2 = []
                for f in range(4):
                    p = ps2_pool.tile([128, H], F32, tag=f"ps2_{f}")
                    gmat = G2sb if f == 3 else Gsb
                    parts = []
                    if wb > 0:
                        parts.append((wb - 1, 64, 128))
                    parts.append((wb, 0, 128))
                    if wb < NT - 1:
                        parts.append((wb + 1, 0, 32))
                    for j, (wt, r0, r1) in enumerate(parts):
                        nc.tensor.matmul(
                            p[:, :],
                            lhsT=gmat[wt][r0:r1, 128 * wb:128 * (wb + 1)],
                            rhs=o1[f][wt][r0:r1, :],
                            start=(j == 0), stop=(j == len(parts) - 1))
                    ps2.append(p)
                Xc, Yc, Sc, XY2c = ps2

                Ycb = tmp_pool.tile([128, H], BF16, tag="Ycb")
                nc.vector.tensor_copy(Ycb[:], Yc[:])
                P = tmp_pool.tile([128, H], BF16, tag="P")
                nc.vector.tensor_tensor(P[:], Xc[:], Ycb[:], ALU.mult)
                A = tmp_pool.tile([128, H], BF16, tag="A")
                nc.scalar.activation(A[:], Xc[:], AF.Square)
                Bt = tmp_pool.tile([128, H], BF16, tag="B")
                nc.scalar.activation(Bt[:], Ycb[:], AF.Square)
                AB = tmp_pool.tile([128, H], BF16, tag="AB")
                nc.vector.tensor_tensor(AB[:], A[:], Bt[:], ALU.add)

                P2 = tmp_pool.tile([128, H], BF16, tag="P2")
                nc.vector.tensor_scalar_mul(P2[:], P[:], 2.0)
                n1 = tmp_pool.tile([128, H], BF16, tag="n1")
                nc.vector.tensor_scalar_add(n1[:], P2[:], C1)
                Xp = tmp_pool.tile([128, H], BF16, tag="Xp")
                nc.vector.tensor_scalar_add(Xp[:], XY2c[:], C2)
                n2 = tmp_pool.tile([128, H], BF16, tag="n2")
                nc.vector.tensor_tensor(n2[:], Xp[:], P2[:], ALU.subtract)
                num = tmp_pool.tile([128, H], BF16, tag="num")
                nc.vector.tensor_tensor(num[:], n1[:], n2[:], ALU.mult)

                den1 = tmp_pool.tile([128, H], BF16, tag="den1")
                nc.vector.tensor_scalar_add(den1[:], AB[:], C1)
                Scp = tmp_pool.tile([128, H], BF16, tag="Scp")
                nc.vector.tensor_scalar_add(Scp[:], Sc[:], C2)
                den2 = tmp_pool.tile([128, H], BF16, tag="den2")
                nc.vector.tensor_tensor(den2[:], Scp[:], AB[:], ALU.subtract)
                den = tmp_pool.tile([128, H], BF16, tag="den")
                nc.vector.tensor_tensor(den[:], den1[:], den2[:], ALU.mult)

                rden = tmp_pool.tile([128, H], F32, tag="rden")
                nc.vector.reciprocal(rden[:], den[:])
                ssim = tmp_pool.tile([128, H], BF16, tag="ssim")
                nc.vector.tensor_tensor(ssim[:], num[:], rden[:], ALU.mult)

                nc.tensor.matmul(acc_ssim[:, :], lhsT=ones[:], rhs=ssim[:],
                                 start=(i_ssim == 0), stop=(i_ssim == n_ssim - 1),
                                 skip_group_check=True)
                i_ssim += 1

        for j, accp in enumerate([acc_ssim, acc_s, acc_xy]):
            stage = const_pool.tile([1, H], F32, tag=f"stage{j}")
            nc.scalar.copy(stage[:], accp[:])
            nc.gpsimd.dma_start(out_ext[j:j + 1, :], stage[:])
    nc.compile()
    return nc


_NC_CACHE = None


def _get_nc():
    global _NC_CACHE
    if _NC_CACHE is None:
        _NC_CACHE = build_nc()
    return _NC_CACHE


last_exec_time_ns = None


def kernel(recon, original, _trace=False):
    global last_exec_time_ns
    recon = np.ascontiguousarray(np.asarray(recon, dtype=np.float32))
    original = np.ascontiguousarray(np.asarray(original, dtype=np.float32))
    G = _band_matrix()

    per = B // NCORES
    in_maps = []
    for c in range(NCORES):
        in_maps.append({
            "x": recon[c * per:(c + 1) * per].reshape(NIMG, H, W),
            "y": original[c * per:(c + 1) * per].reshape(NIMG, H, W),
            "g": G,
        })

    nc = _get_nc()
    res = run_bass_kernel_spmd(nc, in_maps, list(range(NCORES)), trace=_trace)
    last_exec_time_ns = res.exec_time_ns

    n_total = float(B * C * H * W)
    s_ssim = s_s = s_xy = 0.0
    for c in range(NCORES):
        out = np.asarray(res.results[c]["out"], dtype=np.float64)
        s_ssim += out[0].sum()
        s_s += out[1].sum()
        s_xy += out[2].sum()

    mse = (s_s - 2.0 * s_xy) / n_total
    ssim_mean = s_ssim / n_total
    loss = MSE_W * mse + SSIM_W * (1.0 - ssim_mean)
    return np.float32(loss)



# revision 5
# speedup vs baseline: 1.7043x; 1.7043x over previous
"""MSE + SSIM combined loss on Trainium2, data-parallel over 8 NeuronCores.

Reference computes, over [64,3,512,512] f32 inputs:
    loss = 0.7*mean((x-y)^2) + 0.3*(1 - mean(ssim_map(x, y)))
with an 11x11 gaussian (sigma=1.5) depthwise conv, zero-padded (pad=5).

Per core (8 images = 24 channel-images of [512,512]):
  - cast-during-DMA loads: xb, yb [128, 4*512] bf16 (partition p holds rows
    {p, 128+p, 256+p, 384+p})
  - prep: x2,y2 = Square on ACT; s = x2+y2 and xy = xb*yb via DVE
    tensor_tensor_reduce which also emits per-partition sums (for MSE)
  - separable gaussian conv as two banded matmul passes on TensorE:
      d1 (h-conv, transposing): ps1[w_blk, h] += X[h'_blk, w_blk]^T G[h'_blk, band]
      d2 (w-conv): M[wb, h] += G[wt, wb]^T o1[wt, h]; x and xy fields use 2G
        so the PSUM results are M1=2*mu1, XY=2*conv(xy) directly
  - PSUM evacuations all on ACT with folded scale/bias:
      a1=M1, a2=M2, q1=Square(0.5*M1)=mu1^2, q2=mu2^2, xc=XY+C2, sc=S+C1+C2
  - ssim elementwise per image on [128, 2048] bf16 tiles:
      P2=a1*a2 (=2 mu1 mu2); num=(P2+C1)*(xc-P2); den1=(q1+C1)+q2;
      den=den1*(sc-den1); ssim=num*recip(den), summed via tensor_tensor_reduce
  - host combines the [128, 24] per-partition partial sums
"""

import numpy as np
from contextlib import ExitStack

import concourse.bass as bass
import concourse.bacc as bacc
import concourse.mybir as mybir
from concourse import tile
from concourse.bass_utils import run_bass_kernel_spmd

F32 = mybir.dt.float32
BF16 = mybir.dt.bfloat16
AF = mybir.ActivationFunctionType
ALU = mybir.AluOpType

# ---- problem constants (hardcoded; kernel.py must be self-contained) ----
WIN = 11
SIGMA = 1.5
PAD = WIN // 2
DATA_RANGE = 2.0
MSE_W = 0.7
SSIM_W = 0.3
C1 = (0.01 * DATA_RANGE) ** 2
C2 = (0.03 * DATA_RANGE) ** 2

B, C, H, W = 64, 3, 512, 512
NCORES = 8
NIMG = (B // NCORES) * C      # 24 channel-images per core
NT = H // 128                 # 4 tiles per image dim
FD = NT * W                   # 2048 free-dim for per-image tiles


def _gauss1d():
    coords = np.arange(WIN, dtype=np.float64) - (WIN - 1) / 2.0
    g = np.exp(-(coords ** 2) / (2.0 * SIGMA ** 2))
    return (g / g.sum()).astype(np.float32)


def _band_matrix():
    """G[i, j] = g1d[j - i + PAD] for |j-i|<=PAD else 0  (512x512 f32)."""
    g = _gauss1d()
    G = np.zeros((H, H), dtype=np.float32)
    for d in range(-PAD, PAD + 1):
        np.fill_diagonal(G[max(0, -d):, max(0, d):], g[d + PAD])
    return G


def _band(k):
    """Nonzero output-column range of G rows [128k, 128k+128)."""
    return max(0, 128 * k - PAD), min(H, 128 * (k + 1) + PAD)


def build_nc():
    nc = bacc.Bacc("TRN2")
    x_ext = nc.declare_dram_parameter("x", [NIMG, NT, 128, W], F32, isOutput=False)
    y_ext = nc.declare_dram_parameter("y", [NIMG, NT, 128, W], F32, isOutput=False)
    g_ext = nc.declare_dram_parameter("g", [NT, 128, H], F32, isOutput=False)
    g2_ext = nc.declare_dram_parameter("g2", [NT, 128, H], F32, isOutput=False)
    # per-partition partial sums: [0]=ssim, [1]=s, [2]=xy
    out_ext = nc.declare_dram_parameter("out", [3, 128, NIMG], F32, isOutput=True)

    with ExitStack() as ctx:
        tc = ctx.enter_context(tile.TileContext(nc))
        const_pool = ctx.enter_context(tc.tile_pool(name="const", bufs=1))
        in_pool = ctx.enter_context(tc.tile_pool(name="inp", bufs=3))
        fld_pool = ctx.enter_context(tc.tile_pool(name="fld", bufs=2))
        o1_pool = ctx.enter_context(tc.tile_pool(name="o1", bufs=2))
        ev_pool = ctx.enter_context(tc.tile_pool(name="ev", bufs=2))
        ew_pool = ctx.enter_context(tc.tile_pool(name="ew", bufs=1))
        ps1_pool = ctx.enter_context(tc.tile_pool(name="ps1", bufs=3, space="PSUM"))
        ps2_pool = ctx.enter_context(tc.tile_pool(name="ps2", bufs=1, space="PSUM"))

        # ---- constants: G blocks as bf16 (cast during DMA) ----
        Gsb, G2sb = [], []
        for k in range(NT):
            gk = const_pool.tile([128, H], BF16, tag=f"g{k}")
            nc.gpsimd.dma_start(gk[:], g_ext[k])
            g2k = const_pool.tile([128, H], BF16, tag=f"g2{k}")
            nc.gpsimd.dma_start(g2k[:], g2_ext[k])
            Gsb.append(gk)
            G2sb.append(g2k)

        # ---- per-partition accumulators (written column-per-image) ----
        ssacc = const_pool.tile([128, NIMG], F32, tag="ssacc")
        sacc = const_pool.tile([128, NIMG], F32, tag="sacc")
        xyacc = const_pool.tile([128, NIMG], F32, tag="xyacc")

        for i in range(NIMG):
            # ---- load (cast f32 -> bf16 during DMA) ----
            xb = in_pool.tile([128, NT, W], BF16, tag="xb")
            nc.gpsimd.dma_start(xb[:], x_ext[i].rearrange("t p w -> p t w"))
            yb = in_pool.tile([128, NT, W], BF16, tag="yb")
            nc.gpsimd.dma_start(yb[:], y_ext[i].rearrange("t p w -> p t w"))
            xb = xb.rearrange("p t w -> p (t w)")
            yb = yb.rearrange("p t w -> p (t w)")

            # ---- field prep ----
            x2 = fld_pool.tile([128, FD], BF16, tag="x2")
            nc.scalar.activation(x2[:], xb, AF.Square)
            y2 = fld_pool.tile([128, FD], BF16, tag="y2")
            nc.scalar.activation(y2[:], yb, AF.Square)
            s = fld_pool.tile([128, FD], BF16, tag="s")
            nc.vector.scalar_tensor_tensor(
                s[:], x2[:], 0.0, y2[:], ALU.add, ALU.add,
                accum_out=sacc[:, i:i + 1])
            xy = fld_pool.tile([128, FD], BF16, tag="xy")
            nc.vector.scalar_tensor_tensor(
                xy[:], xb, 0.0, yb, ALU.add, ALU.mult,
                accum_out=xyacc[:, i:i + 1])

            fields = [xb, yb, s[:], xy[:]]

            # ---- d1: h-conv, transposing.  o1[f][:, 512wb:] = [w_blk, h] ----
            o1 = []
            for f in range(4):
                o1f = o1_pool.tile([128, FD], BF16, tag=f"o1_{f}")
                o1.append(o1f)
                for wb in range(NT):
                    ps1 = ps1_pool.tile([128, H], F32, tag="ps1")
                    for k in range(NT):
                        # k=0 streams full width so the start=True matmul
                        # writes the whole bank (sim models has_written
                        # per-instruction); G is zero outside the band.
                        lo, hi = (0, H) if k == 0 else _band(k)
                        nc.tensor.matmul(
                            ps1[:, lo:hi],
                            lhsT=fields[f][:, W * k + 128 * wb:W * k + 128 * (wb + 1)],
                            rhs=Gsb[k][:, lo:hi],
                            start=(k == 0), stop=(k == NT - 1),
                            skip_group_check=True)
                    nc.scalar.copy(o1f[:, W * wb:W * (wb + 1)], ps1[:])

            # ---- d2: w-conv + ACT evacuations with folded scale/bias ----
            # field 0 (x) and 3 (xy) use 2G: M1 = 2*mu1, XY = 2*conv(xy)
            a1 = ev_pool.tile([128, FD], BF16, tag="a1")
            a2 = ev_pool.tile([128, FD], BF16, tag="a2")
            q1 = ev_pool.tile([128, FD], BF16, tag="q1")
            q2 = ev_pool.tile([128, FD], BF16, tag="q2")
            xc = ev_pool.tile([128, FD], BF16, tag="xc")
            sc = ev_pool.tile([128, FD], BF16, tag="sc")
            for wb in range(NT):
                parts = []
                if wb > 0:
                    parts.append((wb - 1, 64, 128))
                parts.append((wb, 0, 128))
                if wb < NT - 1:
                    parts.append((wb + 1, 0, 32))
                ps2 = []
                for f in range(4):
                    p = ps2_pool.tile([128, H], F32, tag=f"ps2_{f}")
                    gmat = G2sb if f in (0, 3) else Gsb
                    for j, (wt, r0, r1) in enumerate(parts):
                        nc.tensor.matmul(
                            p[:, :],
                            lhsT=gmat[wt][r0:r1, 128 * wb:128 * (wb + 1)],
                            rhs=o1[f][r0:r1, W * wt:W * (wt + 1)],
                            start=(j == 0), stop=(j == len(parts) - 1))
                    ps2.append(p)
                M1, M2, S, XY = ps2
                sl = slice(W * wb, W * (wb + 1))
                nc.scalar.copy(a1[:, sl], M1[:])
                nc.scalar.copy(a2[:, sl], M2[:])
                nc.scalar.activation(q1[:, sl], M1[:], AF.Square, scale=0.5)
                nc.scalar.activation(q2[:, sl], M2[:], AF.Square)
                nc.scalar.activation(xc[:, sl], XY[:], AF.Copy, bias=C2)
                nc.scalar.activation(sc[:, sl], S[:], AF.Copy, bias=C1 + C2)

            # ---- ssim elementwise on [128, 2048] ----
            P2 = ew_pool.tile([128, FD], BF16, tag="P2")
            nc.vector.tensor_tensor(P2[:], a1[:], a2[:], ALU.mult)
            n2 = ew_pool.tile([128, FD], BF16, tag="n2")
            nc.vector.tensor_tensor(n2[:], xc[:], P2[:], ALU.subtract)
            num = ew_pool.tile([128, FD], BF16, tag="num")
            nc.vector.scalar_tensor_tensor(
                num[:], P2[:], C1, n2[:], ALU.add, ALU.mult)
            den1 = ew_pool.tile([128, FD], BF16, tag="den1")
            nc.vector.scalar_tensor_tensor(
                den1[:], q1[:], C1, q2[:], ALU.add, ALU.add)
            den2 = ew_pool.tile([128, FD], BF16, tag="den2")
            nc.vector.tensor_tensor(den2[:], sc[:], den1[:], ALU.subtract)
            den = ew_pool.tile([128, FD], F32, tag="den")
            nc.vector.tensor_tensor(den[:], den1[:], den2[:], ALU.mult)
            rden = ew_pool.tile([128, FD], F32, tag="rden")
            nc.vector.reciprocal_approx_fast(rden[:], den[:])
            scr = ew_pool.tile([128, FD], BF16, tag="scr")
            nc.vector.scalar_tensor_tensor(
                scr[:], num[:], 0.0, rden[:], ALU.add, ALU.mult,
                accum_out=ssacc[:, i:i + 1])

        nc.gpsimd.dma_start(out_ext[0], ssacc[:])
        nc.gpsimd.dma_start(out_ext[1], sacc[:])
        nc.gpsimd.dma_start(out_ext[2], xyacc[:])
    nc.compile()
    return nc


_NC_CACHE = None


def _get_nc():
    global _NC_CACHE
    if _NC_CACHE is None:
        _NC_CACHE = build_nc()
    return _NC_CACHE


last_exec_time_ns = None


def kernel(recon, original, _trace=False):
    global last_exec_time_ns
    recon = np.ascontiguousarray(np.asarray(recon, dtype=np.float32))
    original = np.ascontiguousarray(np.asarray(original, dtype=np.float32))
    G = _band_matrix()
    G4 = G.reshape(NT, 128, H)
    G24 = (2.0 * G).reshape(NT, 128, H)

    per = B // NCORES
    in_maps = []
    for c in range(NCORES):
        in_maps.append({
            "x": recon[c * per:(c + 1) * per].reshape(NIMG, NT, 128, W),
            "y": original[c * per:(c + 1) * per].reshape(NIMG, NT, 128, W),
            "g": G4,
            "g2": G24,
        })

    nc = _get_nc()
    res = run_bass_kernel_spmd(nc, in_maps, list(range(NCORES)), trace=_trace)
    last_exec_time_ns = res.exec_time_ns

    n_total = float(B * C * H * W)
    s_ssim = s_s = s_xy = 0.0
    for c in range(NCORES):
        out = np.asarray(res.results[c]["out"], dtype=np.float64)
        s_ssim += out[0].sum()
        s_s += out[1].sum()
        s_xy += out[2].sum()

    mse = (s_s - 2.0 * s_xy) / n_total
    ssim_mean = s_ssim / n_total
    loss = MSE_W * mse + SSIM_W * (1.0 - ssim_mean)
    return np.float32(loss)
